# revision 1
# baseline (speedup 1.0000x reference)
"""Trainium2 Bass kernel for masked dot-product attention.

Problem: B=16, Lq=Lk=2048, d=128, fp32.
  scores = Q @ K^T / sqrt(d); mask key positions >= valid_len with -1e6;
  attn = softmax(scores, axis=-1); out = attn @ V.

Strategy
--------
The work is sharded over (batch, query-quarter): 16 batches x 4 q-chunks of
512 = 64 shards, 8 per core. A shard's device cost is proportional to
ceil(valid_len/128) key tiles, so shards are sorted by tile count and slot s
of every core runs the 8 shards ranked [8s, 8s+8); the compiled program bakes
per-slot key extents E_s = max tile count in that rank band. Device work thus
scales with the actual valid lengths (~2x less than processing all keys),
and every core executes an identical instruction stream (SPMD).

All layout work happens on the host inside kernel():
  * Q^T, K^T ([d, L], d on partitions) are prepared with numpy, so the device
    does zero transposes.
  * Masking is exact and host-side: V rows at k >= valid_len are zeroed and a
    0/1 vector z replaces the "ones" column of the softmax denominator. exp()
    never sees masked scores, so no -1e6 arithmetic happens on device.
  * Matmul operands are pre-rounded to fp32r (11-bit mantissa) on the host so
    the PE streams 1 column/cycle (plain fp32 is 4x slower).

Device program per slot (one 512-wide q-chunk, E_s key tiles):
  MM1:  S^T[k,q] = (K^T tile).T @ Q^T     (PE, fp32r, N=512, k-tile stationary)
  exp:  E = exp(S^T / sqrt(d))            (ACT, PSUM->SBUF fp32r, 2 k-tiles/pass)
  MM2:  num^T[d,q] += lhsT=V_tile[k,d] -> sum_k V[k,d]*E[k,q]   (PE accumulate)
  den:  den[q]     += z_tile.T @ E        (PE, lhsT=[128,1])
MM1/exp run one k-group ahead of MM2/den (software pipeline; psum double
buffered) so PE and ACT overlap. Host computes out = (num^T / den).T per
shard. Softmax needs no max-subtraction: scores ~ N(0,1), exp() cannot
overflow fp32, and masked columns contribute exactly zero.
"""

import math

import numpy as np

B, L, D = 16, 2048, 128
NCORES = 8
QCHUNK = 512
NQCHUNKS = L // QCHUNK
NSLOTS = B * NQCHUNKS // NCORES  # 8
GSZ = 2  # k-tiles per exp group; [128, GSZ*512] psum tiles, double-buffered
SCALE = 1.0 / math.sqrt(D)

_programs = {}

# Test hooks: _REPEAT>1 duplicates the whole slot schedule inside one NEFF
# (for wall-clock-delta timing); _last_results holds the raw results.
_TRACE = False
_REPEAT = 1
_last_results = None


def _round_f32r(arr):
    """Round-to-nearest-even fp32 -> fp32r (11-bit mantissa, low 12 bits zero)."""
    bits = np.ascontiguousarray(arr, dtype=np.float32).view(np.uint32)
    keep = bits & np.uint32(0xFFFFF000)
    rem = bits & np.uint32(0x00000FFF)
    lsb = (bits >> np.uint32(12)) & np.uint32(1)
    roundup = (rem > 0x800) | ((rem == 0x800) & (lsb == 1))
    return (keep + (roundup.astype(np.uint32) << np.uint32(12))).view(np.float32)


def _build_program(extents, repeat=1):
    import concourse.tile as tile
    from concourse import bacc, mybir

    F32 = mybir.dt.float32
    F32R = mybir.dt.float32r
    Tmax = max(extents)

    nc = bacc.Bacc("TRN2")

    ins = {}
    outs = {}
    for s, T in enumerate(extents):
        ins[f"qt{s}"] = nc.dram_tensor(f"qt{s}", [128, QCHUNK], F32R, kind="ExternalInput")
        ins[f"kt{s}"] = nc.dram_tensor(f"kt{s}", [128, T * 128], F32R, kind="ExternalInput")
        ins[f"v{s}"] = nc.dram_tensor(f"v{s}", [128, T * 128], F32R, kind="ExternalInput")
        ins[f"z{s}"] = nc.dram_tensor(f"z{s}", [128, T], F32R, kind="ExternalInput")
        outs[f"num{s}"] = nc.dram_tensor(f"num{s}", [128, QCHUNK], F32, kind="ExternalOutput")
        outs[f"den{s}"] = nc.dram_tensor(f"den{s}", [1, QCHUNK], F32, kind="ExternalOutput")

    with tile.TileContext(nc) as tc:
        with (
            tc.tile_pool(name="inp", bufs=3) as inp,
            tc.tile_pool(name="epool", bufs=3) as epool,
            tc.tile_pool(name="opool", bufs=3) as opool,
            tc.tile_pool(name="dpool", bufs=3) as dpool,
            tc.tile_pool(name="ps_s", bufs=2, space="PSUM") as ps_s,
            tc.tile_pool(name="ps_o", bufs=2, space="PSUM") as ps_o,
            tc.tile_pool(name="ps_d", bufs=2, space="PSUM") as ps_d,
        ):
            for s, T in [(s, T) for _ in range(repeat) for s, T in enumerate(extents)]:
                qt = inp.tile([128, QCHUNK], F32R, tag="qt")
                kt = inp.tile([128, Tmax * 128], F32R, tag="kt")
                vt = inp.tile([128, Tmax * 128], F32R, tag="vt")
                zt = inp.tile([128, Tmax], F32R, tag="zt")
                nc.sync.dma_start(out=kt[:, : T * 128], in_=ins[f"kt{s}"][:, :])
                nc.sync.dma_start(out=qt, in_=ins[f"qt{s}"][:, :])
                nc.sync.dma_start(out=vt[:, : T * 128], in_=ins[f"v{s}"][:, :])
                nc.sync.dma_start(out=zt[:, :T], in_=ins[f"z{s}"][:, :])
                # [128,1]-stationary matmuls are pathologically slow on HW
                # (~70us each), so the denominator matmul uses a full 128-col
                # stationary: z broadcast along the free dim (every output row
                # then holds the denominator; row 0 is copied out).
                zr = inp.tile([128, Tmax * 128], F32R, tag="zr")
                for t in range(T):
                    nc.vector.tensor_copy(
                        zr[:, t * 128 : (t + 1) * 128],
                        zt[:, t : t + 1].to_broadcast([128, 128]),
                    )

                ngroups = (T + GSZ - 1) // GSZ
                po = ps_o.tile([128, QCHUNK], F32, tag="po")
                pd = ps_d.tile([128, QCHUNK], F32, tag="pd")
                # MM1/exp of group g run one group ahead of MM2/den of g-1:
                # the PE issues the next group's MM1 (feeding ACT) before
                # draining the previous group's consumers.
                pending = None
                for g in range(ngroups + 1):
                    if g < ngroups:
                        gtiles = list(range(g * GSZ, min(g * GSZ + GSZ, T)))
                        gn = len(gtiles)
                        pss = ps_s.tile([128, GSZ * QCHUNK], F32, tag="ps")
                        for j, t in enumerate(gtiles):
                            nc.tensor.matmul(
                                pss[:, j * QCHUNK : (j + 1) * QCHUNK],
                                kt[:, t * 128 : (t + 1) * 128],
                                qt,
                                start=True,
                                stop=True,
                            )
                        eg = epool.tile([128, GSZ * QCHUNK], F32R, tag="eg")
                        nc.scalar.activation(
                            eg[:, : gn * QCHUNK],
                            pss[:, : gn * QCHUNK],
                            mybir.ActivationFunctionType.Exp,
                            scale=SCALE,
                        )
                        cur = (gtiles, eg)
                    else:
                        cur = None
                    if pending is not None:
                        ptiles, peg = pending
                        for j, t in enumerate(ptiles):
                            es = peg[:, j * QCHUNK : (j + 1) * QCHUNK]
                            nc.tensor.matmul(
                                po,
                                vt[:, t * 128 : (t + 1) * 128],
                                es,
                                start=(t == 0),
                                stop=(t == T - 1),
                            )
                            nc.tensor.matmul(
                                pd,
                                zr[:, t * 128 : (t + 1) * 128],
                                es,
                                start=(t == 0),
                                stop=(t == T - 1),
                            )
                    pending = cur
                osb = opool.tile([128, QCHUNK], F32, tag="osb")
                nc.vector.tensor_copy(osb, po)
                nc.sync.dma_start(out=outs[f"num{s}"][:, :], in_=osb)
                dsb = dpool.tile([1, QCHUNK], F32, tag="dsb")
                nc.vector.tensor_copy(dsb, pd[0:1, :])
                nc.sync.dma_start(out=outs[f"den{s}"][:, :], in_=dsb)

    nc.finalize()
    return nc


def _get_program(extents, repeat=1):
    key = (tuple(extents), repeat)
    if key not in _programs:
        _programs[key] = _build_program(tuple(extents), repeat)
    return _programs[key]


def _shard_plan(vl):
    """64 (batch, q-chunk) shards sorted by key-tile count desc; slot s of
    core c runs shard rank s*8+c. Returns (shards, extents)."""
    tiles = [max(1, int(math.ceil(int(vl[b]) / 128.0))) for b in range(B)]
    shards = sorted(
        ((tiles[b], b, qc) for b in range(B) for qc in range(NQCHUNKS)),
        key=lambda x: (-x[0], x[1], x[2]),
    )
    extents = tuple(shards[s * NCORES][0] for s in range(NSLOTS))
    return shards, extents


def _make_in_maps(queries, keys, values, vl, shards, extents):
    # kt/vt/zt depend only on (batch, extent): memoize across the 4 q-shards
    kcache = {}

    def kvz(b, T):
        key = (b, T)
        if key not in kcache:
            n = int(vl[b])
            vs = values[b, : T * 128].copy()
            vs[n:] = 0.0
            z = np.zeros((T * 128,), np.float32)
            z[:n] = 1.0
            kcache[key] = (
                _round_f32r(keys[b, : T * 128].T),
                _round_f32r(vs.reshape(T, 128, D).transpose(1, 0, 2).reshape(128, T * D)),
                np.ascontiguousarray(z.reshape(T, 128).T),
            )
        return kcache[key]

    qtr = {}  # rounded Q^T per batch

    def qtb(b):
        if b not in qtr:
            qtr[b] = _round_f32r(queries[b].T)
        return qtr[b]

    in_maps = [{} for _ in range(NCORES)]
    for s in range(NSLOTS):
        T = extents[s]
        for c in range(NCORES):
            _, b, qc = shards[s * NCORES + c]
            kt, vt, zt = kvz(b, T)
            m = in_maps[c]
            m[f"qt{s}"] = np.ascontiguousarray(
                qtb(b)[:, qc * QCHUNK : (qc + 1) * QCHUNK]
            )
            m[f"kt{s}"] = kt
            m[f"v{s}"] = vt
            m[f"z{s}"] = zt
    return in_maps


def kernel(queries, keys, values, valid_lens):
    from concourse.bass_utils import run_bass_kernel_spmd

    queries = np.ascontiguousarray(queries, dtype=np.float32)
    keys = np.ascontiguousarray(keys, dtype=np.float32)
    values = np.ascontiguousarray(values, dtype=np.float32)
    vl = np.asarray(valid_lens).astype(np.int64).clip(1, L)
    assert queries.shape == (B, L, D), queries.shape

    shards, extents = _shard_plan(vl)
    nc = _get_program(extents, _REPEAT)
    in_maps = _make_in_maps(queries, keys, values, vl, shards, extents)

    res = run_bass_kernel_spmd(nc, in_maps, core_ids=list(range(NCORES)), trace=_TRACE)
    globals()["_last_results"] = res

    out = np.empty((B, L, D), np.float32)
    for s in range(NSLOTS):
        for c in range(NCORES):
            _, b, qc = shards[s * NCORES + c]
            r = res.results[c]
            num = r[f"num{s}"]  # [128, QCHUNK]
            den = r[f"den{s}"]  # [1, QCHUNK]
            out[b, qc * QCHUNK : (qc + 1) * QCHUNK] = (num / den).T
    return out



# revision 19
# speedup vs baseline: 1.7279x; 1.7279x over previous
"""Trainium2 Bass kernel for masked dot-product attention (v2).

Problem: B=16, Lq=Lk=2048, d=128, fp32.
  scores = Q @ K^T / sqrt(d); mask key positions >= valid_len with -1e6;
  attn = softmax(scores, axis=-1); out = attn @ V.

Strategy
--------
Work is sharded over (batch, query-half): 16 batches x 2 q-chunks of 1024 =
32 shards, 4 per core. A shard's cost is proportional to ceil(valid_len/128)
key tiles, so shards are sorted by tile count and slot s of every core runs
the 8 shards ranked [8s, 8s+8); the compiled program bakes per-slot key
extents E_s = max tile count in that rank band. Every core executes an
identical instruction stream (SPMD).

All layout work happens on the host inside kernel():
  * Q^T, K^T ([d, L], d on partitions) and V tile-blocks are prepared with
    numpy in bfloat16 (PE streams 1 column/cycle; DMA bytes halved).
  * Masking is exact and host-side: V rows at k >= valid_len are zeroed, and
    the denominator matmul's stationary for the LAST key tile is a host-built
    0/1 matrix zb ([128,128], zb[k,:] = k valid); all earlier tiles use a
    memset all-ones stationary. exp() output of masked columns never reaches
    the numerator (V rows zero) or denominator (zb zero).

Device program per slot (one 1024-wide q-chunk, E_s key tiles):
  MM1:  S^T[k,q] = (K^T tile).T @ Q^T     (PE, bf16, one 1024-col matmul)
  exp:  E = exp(S^T / sqrt(d))            (ACT, PSUM->SBUF bf16)
  MM2:  num^T[d,q] += V_tile[k,d].T-blocks @ E   (PE accumulate)
  den:  den[q]     += (ones|zb).T @ E     (PE accumulate; every row = den)
MM1/exp of tile t run one tile ahead of MM2/den of t-1 (software pipeline,
psum double-buffered) so PE and ACT overlap. Input DMAs are split across the
two HWDGE queues (SP: kt,qt; ACT: vt,zb). Host computes
out = (num^T / den).T per shard. Softmax needs no max-subtraction: scores
~ N(0,1), exp cannot overflow, masked columns contribute exactly zero.
"""

import math

import numpy as np

B, L, D = 16, 2048, 128
NCORES = 8
QCHUNK = 1024
NQCHUNKS = L // QCHUNK
NSLOTS = B * NQCHUNKS // NCORES  # 4
SCALE = 1.0 / math.sqrt(D)

_programs = {}

_TRACE = False
_REPEAT = 1
_last_results = None


def _bf16(arr):
    import ml_dtypes

    return np.ascontiguousarray(arr, dtype=np.float32).astype(ml_dtypes.bfloat16)


def _build_program(extents, repeat=1, ablate=()):
    import concourse.tile as tile
    from concourse import bacc, mybir

    ablate = set(ablate)
    F32 = mybir.dt.float32
    BF16 = mybir.dt.bfloat16
    Tmax = max(extents)

    nc = bacc.Bacc("TRN2")

    ins = {}
    outs = {}
    for s, T in enumerate(extents):
        ins[f"qt{s}"] = nc.dram_tensor(f"qt{s}", [128, QCHUNK], BF16, kind="ExternalInput")
        ins[f"kt{s}"] = nc.dram_tensor(f"kt{s}", [128, T * 128], BF16, kind="ExternalInput")
        ins[f"v{s}"] = nc.dram_tensor(f"v{s}", [128, T * 128], BF16, kind="ExternalInput")
        outs[f"num{s}"] = nc.dram_tensor(f"num{s}", [128, QCHUNK], BF16, kind="ExternalOutput")
        outs[f"den{s}"] = nc.dram_tensor(f"den{s}", [1, QCHUNK], F32, kind="ExternalOutput")

    with tile.TileContext(nc) as tc:
        with (
            tc.tile_pool(name="cpool", bufs=1) as cpool,
            tc.tile_pool(name="inp", bufs=3) as inp,
            tc.tile_pool(name="epool", bufs=3) as epool,
            tc.tile_pool(name="opool", bufs=2) as opool,
            tc.tile_pool(name="dpool", bufs=2) as dpool,
            tc.tile_pool(name="ps_s", bufs=2, space="PSUM") as ps_s,
            tc.tile_pool(name="ps_o", bufs=1, space="PSUM") as ps_o,
            tc.tile_pool(name="ps_d", bufs=1, space="PSUM") as ps_d,
        ):
            ones = cpool.tile([128, 128], BF16, tag="ones")
            nc.vector.memset(ones, 1.0)
            for s, T in [(s, T) for _ in range(repeat) for s, T in enumerate(extents)]:
                qt = inp.tile([128, QCHUNK], BF16, tag="qt")
                kt = inp.tile([128, Tmax * 128], BF16, tag="kt")
                vt = inp.tile([128, Tmax * 128], BF16, tag="vt")
                if "dma_in" not in ablate:
                    nc.sync.dma_start(out=kt[:, : T * 128], in_=ins[f"kt{s}"][:, :])
                    nc.sync.dma_start(out=qt, in_=ins[f"qt{s}"][:, :])
                    nc.scalar.dma_start(out=vt[:, : T * 128], in_=ins[f"v{s}"][:, :])
                else:
                    nc.sync.dma_start(out=kt[:, :128], in_=ins[f"kt{s}"][:, :128])
                    nc.sync.dma_start(out=qt[:, :128], in_=ins[f"qt{s}"][:, :128])
                    nc.scalar.dma_start(out=vt[:, :128], in_=ins[f"v{s}"][:, :128])

                po = pd = None
                if "mm2" not in ablate:
                    po = ps_o.tile([128, QCHUNK], F32, tag="po")
                if "den" not in ablate:
                    pd = ps_d.tile([128, QCHUNK], F32, tag="pd")
                # MM1/exp of tile t run one tile ahead of MM2/den of t-1
                pending = None
                for t in range(T + 1 if "mm1" not in ablate else 0):
                    if t < T:
                        pss = ps_s.tile([128, QCHUNK], F32, tag="ps")
                        for h in range(2):  # matmul moving dim caps at 512
                            nc.tensor.matmul(
                                pss[:, h * 512 : (h + 1) * 512],
                                kt[:, t * 128 : (t + 1) * 128],
                                qt[:, h * 512 : (h + 1) * 512],
                                start=True,
                                stop=True,
                            )
                        eg = None
                        if "act" not in ablate:
                            eg = epool.tile([128, QCHUNK], BF16, tag="eg")
                            nc.scalar.activation(
                                eg,
                                pss,
                                mybir.ActivationFunctionType.Exp,
                                scale=SCALE,
                            )
                        cur = (t, eg)
                    else:
                        cur = None
                    if pending is not None and pending[1] is not None:
                        pt, peg = pending
                        for h in range(2):
                            hs = slice(h * 512, (h + 1) * 512)
                            if "mm2" not in ablate:
                                nc.tensor.matmul(
                                    po[:, hs],
                                    vt[:, pt * 128 : (pt + 1) * 128],
                                    peg[:, hs],
                                    start=(pt == 0),
                                    stop=(pt == T - 1),
                                )
                            if "den" not in ablate:
                                nc.tensor.matmul(
                                    pd[:, hs],
                                    ones,
                                    peg[:, hs],
                                    start=(pt == 0),
                                    stop=(pt == T - 1),
                                )
                    pending = cur
                if "sink" in ablate:
                    dsb = dpool.tile([1, QCHUNK], F32, tag="dsb")
                    nc.vector.tensor_copy(dsb[:, 0:128], kt[0:1, 0:128])
                    nc.vector.tensor_copy(dsb[:, 128:256], qt[0:1, 0:128])
                    nc.vector.tensor_copy(dsb[:, 256:384], vt[0:1, 0:128])
                    nc.sync.dma_start(out=outs[f"den{s}"][:, :], in_=dsb)
                if "out" not in ablate:
                    if po is not None:
                        osb = opool.tile([128, QCHUNK], BF16, tag="osb")
                        nc.vector.tensor_copy(osb, po)
                        nc.sync.dma_start(out=outs[f"num{s}"][:, :], in_=osb)
                    if pd is not None:
                        dsb = dpool.tile([1, QCHUNK], F32, tag="dsb")
                        nc.vector.tensor_copy(dsb, pd[0:1, :])
                        nc.scalar.dma_start(out=outs[f"den{s}"][:, :], in_=dsb)

    nc.finalize()
    return nc


def _get_program(extents, repeat=1, ablate=()):
    key = (tuple(extents), repeat, tuple(sorted(ablate)))
    if key not in _programs:
        _programs[key] = _build_program(tuple(extents), repeat, ablate)
    return _programs[key]


def _shard_plan(vl):
    """32 (batch, q-chunk) shards sorted by key-tile count desc; slot s of
    core c runs shard rank s*8+c. Returns (shards, extents)."""
    tiles = [max(1, int(math.ceil(int(vl[b]) / 128.0))) for b in range(B)]
    shards = sorted(
        ((tiles[b], b, qc) for b in range(B) for qc in range(NQCHUNKS)),
        key=lambda x: (-x[0], x[1], x[2]),
    )
    extents = tuple(shards[s * NCORES][0] for s in range(NSLOTS))
    return shards, extents


def _make_in_maps(queries, keys, values, vl, shards, extents):
    # kt/vt depend only on (batch, extent): memoize across the q-shards.
    # K columns at k >= valid_len are zeroed: their scores are exactly 0, so
    # exp contributes exactly 1.0 to the denominator per masked column; the
    # host subtracts that known count after the run. V rows at k >= valid_len
    # are zeroed so masked columns never reach the numerator.
    kcache = {}

    def kv(b, T):
        key = (b, T)
        if key not in kcache:
            n = int(vl[b])
            vs = values[b, : T * 128].copy()
            vs[n:] = 0.0
            ks = keys[b, : T * 128].copy()
            ks[n:] = 0.0
            kcache[key] = (
                _bf16(ks.T),
                _bf16(vs.reshape(T, 128, D).transpose(1, 0, 2).reshape(128, T * D)),
            )
        return kcache[key]

    qtr = {}  # Q^T per batch, bf16

    def qtb(b):
        if b not in qtr:
            qtr[b] = _bf16(queries[b].T)
        return qtr[b]

    in_maps = [{} for _ in range(NCORES)]
    for s in range(NSLOTS):
        T = extents[s]
        for c in range(NCORES):
            _, b, qc = shards[s * NCORES + c]
            kt, vt = kv(b, T)
            m = in_maps[c]
            m[f"qt{s}"] = np.ascontiguousarray(
                qtb(b)[:, qc * QCHUNK : (qc + 1) * QCHUNK]
            )
            m[f"kt{s}"] = kt
            m[f"v{s}"] = vt
    return in_maps


def kernel(queries, keys, values, valid_lens):
    from concourse.bass_utils import run_bass_kernel_spmd

    queries = np.ascontiguousarray(queries, dtype=np.float32)
    keys = np.ascontiguousarray(keys, dtype=np.float32)
    values = np.ascontiguousarray(values, dtype=np.float32)
    vl = np.asarray(valid_lens).astype(np.int64).clip(1, L)
    assert queries.shape == (B, L, D), queries.shape

    shards, extents = _shard_plan(vl)
    nc = _get_program(extents, _REPEAT)
    in_maps = _make_in_maps(queries, keys, values, vl, shards, extents)

    res = run_bass_kernel_spmd(nc, in_maps, core_ids=list(range(NCORES)), trace=_TRACE)
    globals()["_last_results"] = res

    out = np.empty((B, L, D), np.float32)
    for s in range(NSLOTS):
        T = extents[s]
        for c in range(NCORES):
            _, b, qc = shards[s * NCORES + c]
            r = res.results[c]
            num = np.asarray(r[f"num{s}"], dtype=np.float32)  # [128, QCHUNK]
            den = np.asarray(r[f"den{s}"], dtype=np.float32)  # [1, QCHUNK]
            den = den - float(T * 128 - int(vl[b]))  # masked cols added exp(0)=1
            out[b, qc * QCHUNK : (qc + 1) * QCHUNK] = (num / den).T
    return out


# revision 24
# speedup vs baseline: 8.5814x; 4.9665x over previous
"""Trainium2 Bass kernel for masked dot-product attention (v2).

Problem: B=16, Lq=Lk=2048, d=128, fp32.
  scores = Q @ K^T / sqrt(d); mask key positions >= valid_len with -1e6;
  attn = softmax(scores, axis=-1); out = attn @ V.

Strategy
--------
Work is sharded over (batch, query-half): 16 batches x 2 q-chunks of 1024 =
32 shards, 4 per core. A shard's cost is proportional to ceil(valid_len/128)
key tiles, so shards are sorted by tile count and slot s of every core runs
the 8 shards ranked [8s, 8s+8); the compiled program bakes per-slot key
extents E_s = max tile count in that rank band. Every core executes an
identical instruction stream (SPMD).

All layout work happens on the host inside kernel():
  * Q^T, K^T ([d, L], d on partitions) and V tile-blocks are prepared with
    numpy in bfloat16 (fp32r matmuls measured ~8x slower than bf16 on this
    hardware; bf16 also halves DMA bytes). bf16 rel-err lands at ~5e-3,
    well under the 2e-2 gate.
  * Masking is exact and host-side: K columns AND V rows at k >= valid_len
    are zeroed. A zeroed K column makes its score exactly 0, so exp
    contributes exactly 1.0 to the denominator, which the host subtracts as
    a known count afterwards; zeroed V rows keep masked columns out of the
    numerator. No mask tensors or broadcasts on device.

Device program per slot (one 1024-wide q-chunk, E_s key tiles):
  MM1:  S^T[k,q] = (K^T tile).T @ Q^T     (PE, bf16, 2x512-col matmuls)
  exp:  E = exp(S^T / sqrt(d))            (ACT, PSUM->SBUF bf16)
  MM2:  num^T[d,q] += V_tile[k,d].T-blocks @ E   (PE accumulate)
  den:  den[q]     += ones.T @ E          (PE accumulate; every row = den)
MM1/exp of tile t run PIPE=2 tiles ahead of MM2/den of t-2 (software
pipeline, psum double-buffered) so the PE never waits on the ~1-2us
PE->ACT->PE semaphore round-trip. Input DMAs are split across the two HWDGE
queues (SP: kt,qt; ACT: vt) and prefetch up to 4 slots ahead. Host computes
out = (num^T / (den - masked_count)).T per shard. Softmax needs no
max-subtraction: scores ~ N(0,1), exp cannot overflow bf16, and masked
columns are subtracted exactly.
"""

import math

import numpy as np

B, L, D = 16, 2048, 128
NCORES = 8
QCHUNK = 1024
NQCHUNKS = L // QCHUNK
NSLOTS = B * NQCHUNKS // NCORES  # 4
SCALE = 1.0 / math.sqrt(D)

_programs = {}

_TRACE = False
_REPEAT = 1
_last_results = None


def _bf16(arr):
    import ml_dtypes

    return np.ascontiguousarray(arr, dtype=np.float32).astype(ml_dtypes.bfloat16)


def _build_program(extents, repeat=1, ablate=()):
    import concourse.tile as tile
    from concourse import bacc, mybir

    ablate = set(ablate)
    F32 = mybir.dt.float32
    BF16 = mybir.dt.bfloat16
    Tmax = max(extents)

    nc = bacc.Bacc("TRN2")

    ins = {}
    outs = {}
    for s, T in enumerate(extents):
        ins[f"qt{s}"] = nc.dram_tensor(f"qt{s}", [128, QCHUNK], BF16, kind="ExternalInput")
        ins[f"kt{s}"] = nc.dram_tensor(f"kt{s}", [128, T * 128], BF16, kind="ExternalInput")
        ins[f"v{s}"] = nc.dram_tensor(f"v{s}", [128, T * 128], BF16, kind="ExternalInput")
        outs[f"num{s}"] = nc.dram_tensor(f"num{s}", [128, QCHUNK], BF16, kind="ExternalOutput")
        outs[f"den{s}"] = nc.dram_tensor(f"den{s}", [1, QCHUNK], F32, kind="ExternalOutput")

    with tile.TileContext(nc) as tc:
        with (
            tc.tile_pool(name="cpool", bufs=1) as cpool,
            tc.tile_pool(name="inp", bufs=4) as inp,
            tc.tile_pool(name="epool", bufs=4) as epool,
            tc.tile_pool(name="opool", bufs=2) as opool,
            tc.tile_pool(name="dpool", bufs=2) as dpool,
            tc.tile_pool(name="ps_s", bufs=2, space="PSUM") as ps_s,
            tc.tile_pool(name="ps_o", bufs=1, space="PSUM") as ps_o,
            tc.tile_pool(name="ps_d", bufs=1, space="PSUM") as ps_d,
        ):
            ones = cpool.tile([128, 128], BF16, tag="ones")
            nc.vector.memset(ones, 1.0)
            for s, T in [(s, T) for _ in range(repeat) for s, T in enumerate(extents)]:
                qt = inp.tile([128, QCHUNK], BF16, tag="qt")
                kt = inp.tile([128, Tmax * 128], BF16, tag="kt")
                vt = inp.tile([128, Tmax * 128], BF16, tag="vt")
                if "dma_in" not in ablate:
                    nc.sync.dma_start(out=kt[:, : T * 128], in_=ins[f"kt{s}"][:, :])
                    nc.sync.dma_start(out=qt, in_=ins[f"qt{s}"][:, :])
                    nc.scalar.dma_start(out=vt[:, : T * 128], in_=ins[f"v{s}"][:, :])
                else:
                    nc.sync.dma_start(out=kt[:, :128], in_=ins[f"kt{s}"][:, :128])
                    nc.sync.dma_start(out=qt[:, :128], in_=ins[f"qt{s}"][:, :128])
                    nc.scalar.dma_start(out=vt[:, :128], in_=ins[f"v{s}"][:, :128])

                po = pd = None
                if "mm2" not in ablate:
                    po = ps_o.tile([128, QCHUNK], F32, tag="po")
                if "den" not in ablate:
                    pd = ps_d.tile([128, QCHUNK], F32, tag="pd")
                # MM1/exp of tile t run PIPE tiles ahead of MM2/den of
                # t-PIPE, so the PE never waits on the PE->ACT->PE semaphore
                # round-trip of the current tile.
                PIPE = 2
                pending = []
                for t in range(T + PIPE if "mm1" not in ablate else 0):
                    if t < T:
                        pss = ps_s.tile([128, QCHUNK], F32, tag="ps")
                        for h in range(2):  # matmul moving dim caps at 512
                            nc.tensor.matmul(
                                pss[:, h * 512 : (h + 1) * 512],
                                kt[:, t * 128 : (t + 1) * 128],
                                qt[:, h * 512 : (h + 1) * 512],
                                start=True,
                                stop=True,
                            )
                        eg = None
                        if "act" not in ablate:
                            eg = epool.tile([128, QCHUNK], BF16, tag="eg")
                            nc.scalar.activation(
                                eg,
                                pss,
                                mybir.ActivationFunctionType.Exp,
                                scale=SCALE,
                            )
                        pending.append((t, eg))
                    if t >= PIPE and pending:
                        pt, peg = pending.pop(0)
                        if peg is None:
                            continue
                        for h in range(2):
                            hs = slice(h * 512, (h + 1) * 512)
                            if "mm2" not in ablate:
                                nc.tensor.matmul(
                                    po[:, hs],
                                    vt[:, pt * 128 : (pt + 1) * 128],
                                    peg[:, hs],
                                    start=(pt == 0),
                                    stop=(pt == T - 1),
                                )
                            if "den" not in ablate:
                                nc.tensor.matmul(
                                    pd[:, hs],
                                    ones,
                                    peg[:, hs],
                                    start=(pt == 0),
                                    stop=(pt == T - 1),
                                )
                if "sink" in ablate:
                    dsb = dpool.tile([1, QCHUNK], F32, tag="dsb")
                    nc.vector.tensor_copy(dsb[:, 0:128], kt[0:1, 0:128])
                    nc.vector.tensor_copy(dsb[:, 128:256], qt[0:1, 0:128])
                    nc.vector.tensor_copy(dsb[:, 256:384], vt[0:1, 0:128])
                    nc.sync.dma_start(out=outs[f"den{s}"][:, :], in_=dsb)
                if "out" not in ablate:
                    if po is not None:
                        osb = opool.tile([128, QCHUNK], BF16, tag="osb")
                        nc.vector.tensor_copy(osb, po)
                        nc.sync.dma_start(out=outs[f"num{s}"][:, :], in_=osb)
                    if pd is not None:
                        dsb = dpool.tile([1, QCHUNK], F32, tag="dsb")
                        nc.vector.tensor_copy(dsb, pd[0:1, :])
                        nc.scalar.dma_start(out=outs[f"den{s}"][:, :], in_=dsb)

    nc.finalize()
    return nc


def _get_program(extents, repeat=1, ablate=()):
    key = (tuple(extents), repeat, tuple(sorted(ablate)))
    if key not in _programs:
        _programs[key] = _build_program(tuple(extents), repeat, ablate)
    return _programs[key]


def _shard_plan(vl):
    """32 (batch, q-chunk) shards sorted by key-tile count desc; slot s of
    core c runs shard rank s*8+c. Returns (shards, extents)."""
    tiles = [max(1, int(math.ceil(int(vl[b]) / 128.0))) for b in range(B)]
    shards = sorted(
        ((tiles[b], b, qc) for b in range(B) for qc in range(NQCHUNKS)),
        key=lambda x: (-x[0], x[1], x[2]),
    )
    extents = tuple(shards[s * NCORES][0] for s in range(NSLOTS))
    return shards, extents


def _make_in_maps(queries, keys, values, vl, shards, extents):
    # kt/vt depend only on (batch, extent): memoize across the q-shards.
    # K columns at k >= valid_len are zeroed: their scores are exactly 0, so
    # exp contributes exactly 1.0 to the denominator per masked column; the
    # host subtracts that known count after the run. V rows at k >= valid_len
    # are zeroed so masked columns never reach the numerator.
    kcache = {}

    def kv(b, T):
        key = (b, T)
        if key not in kcache:
            n = int(vl[b])
            vs = values[b, : T * 128].copy()
            vs[n:] = 0.0
            ks = keys[b, : T * 128].copy()
            ks[n:] = 0.0
            kcache[key] = (
                _bf16(ks.T),
                _bf16(vs.reshape(T, 128, D).transpose(1, 0, 2).reshape(128, T * D)),
            )
        return kcache[key]

    qtr = {}  # Q^T per batch, bf16

    def qtb(b):
        if b not in qtr:
            qtr[b] = _bf16(queries[b].T)
        return qtr[b]

    in_maps = [{} for _ in range(NCORES)]
    for s in range(NSLOTS):
        T = extents[s]
        for c in range(NCORES):
            _, b, qc = shards[s * NCORES + c]
            kt, vt = kv(b, T)
            m = in_maps[c]
            m[f"qt{s}"] = np.ascontiguousarray(
                qtb(b)[:, qc * QCHUNK : (qc + 1) * QCHUNK]
            )
            m[f"kt{s}"] = kt
            m[f"v{s}"] = vt
    return in_maps


def kernel(queries, keys, values, valid_lens):
    from concourse.bass_utils import run_bass_kernel_spmd

    queries = np.ascontiguousarray(queries, dtype=np.float32)
    keys = np.ascontiguousarray(keys, dtype=np.float32)
    values = np.ascontiguousarray(values, dtype=np.float32)
    vl = np.asarray(valid_lens).astype(np.int64).clip(1, L)
    assert queries.shape == (B, L, D), queries.shape

    shards, extents = _shard_plan(vl)
    nc = _get_program(extents, _REPEAT)
    in_maps = _make_in_maps(queries, keys, values, vl, shards, extents)

    res = run_bass_kernel_spmd(nc, in_maps, core_ids=list(range(NCORES)), trace=_TRACE)
    globals()["_last_results"] = res

    out = np.empty((B, L, D), np.float32)
    for s in range(NSLOTS):
        T = extents[s]
        for c in range(NCORES):
            _, b, qc = shards[s * NCORES + c]
            r = res.results[c]
            num = np.asarray(r[f"num{s}"], dtype=np.float32)  # [128, QCHUNK]
            den = np.asarray(r[f"den{s}"], dtype=np.float32)  # [1, QCHUNK]
            den = den - float(T * 128 - int(vl[b]))  # masked cols added exp(0)=1
            out[b, qc * QCHUNK : (qc + 1) * QCHUNK] = (num / den).T
    return out
